# revision 84
# baseline (speedup 1.0000x reference)
"""MetaLSTMCell TRN2 kernel: pure batch-parallel across 8 cores, no collectives.

Each core owns 256 batch rows (2 tiles of 128) and computes the full hidden
dim (4 gates x 1024 cols, as 8 chunks of 512), so the per-gate LayerNorm is
entirely core-local -- no AllReduce, no CC entry barrier (the measured CC
cost in the old hidden-sharded layout was a 44us entry barrier plus
6-44us per tiny AllReduce).

Host-side weight preprocessing: the hypernetwork projections are folded into
M_*[g] = contract(z*_w[g], d*_w[g]) so that d_*(b) = src_meta[b] @ M_*[g] +
bias row, and all large operands are pre-quantized to fp8-e4m3 (TRN variant,
max +-240) with power-of-2 scales (w_h/w_x x32, M_h/M_x x64, M_b x2048).
y rides at a single global scale SM*SW everywhere; LayerNorm is
scale-invariant so no descales are needed (only eps is rescaled).

GEMMs run as fp8 DoubleRow matmuls (K=256 per instruction -- on this part
DR matches bf16 MAC rate but halves instruction count and weight bytes).
Per-gate bias rows are folded in as K=1 bf16 matmuls against a ones row.
Engine split per 512-col chunk (measured ~0.4-0.7us per [128,512] op on
DVE/ScalarE, ~1.2us on GpSimd, which also cannot read PSUM; DVE may read
at most ONE PSUM operand per instruction):
  ScalarE: stage WH/WX PSUM->SBUF + Square(y) with accum (sum of squares)
  DVE:     u=DH*wh_s, v=DX*wx_s, y=DB+t (PSUM readers), row-sum reduce
  GpSimd:  t=u+v (SBUF-only; switched to DVE for the latency-critical
           last chunks and for the post-GEMM tail where DVE is ~3x faster)
Phase_b per chunk: yn=y*rs+nmrs (tensor_scalar), *lnw (TT), +lnb, activation.

Schedule: chunk 0 runs for both batch tiles back-to-back (the weight
stream is the pacer early on; pairing gives ~10us of tensor work per
~5.6us of weight arrival so the PE array never idles and its HAM clock
gate stays at full rate), then bt0 finishes alone, and bt0's epilogue is
spread under bt1's remaining GEMM slots. Only bt1's epilogue trails.

DMA: weights stream on both HWDGE queues in consumption order (wh on
SyncE, wx on ScalarE, m3 alternating; first chunk k-sliced), activations
first, LN tables last; outputs go out on the (by then idle) sync queue.
Activation tables are preloaded with dummy ops during the ramp.
"""

import sys

sys.path.insert(0, "/opt/trn_rl_repo")

import ml_dtypes
import numpy as np
import concourse.bass as bass
import concourse.mybir as mybir
from concourse.bass_utils import run_bass_kernel_spmd
import concourse.tile as tile

B, IN, H, Z, G = 2048, 1024, 1024, 256, 4
NCORES = 8
BSH = B // NCORES          # 256 batch rows per core
BT = 128                   # batch tile (PE output partitions)
NBT = BSH // BT            # 2 batch tiles per core
CW = 512                   # column chunk width
NC = G * H // CW           # 8 chunks; chunk c = (gate g=c//2, half=c%2)
KC = IN // 256             # 4 DoubleRow K-chunks for the main GEMMs
RS = NC * CW               # 4096: one bias row
SM, SW = 64.0, 32.0        # fp8 pre-scales for M_h/M_x and w_h/w_x
PERM = (0, 1, 3, 2)        # gate order [i, f, o, g]

dt = mybir.dt
AF = mybir.ActivationFunctionType
ALU = mybir.AluOpType
DR = mybir.MatmulPerfMode.DoubleRow
F32, BF16, F8 = dt.float32, dt.bfloat16, dt.float8e4

NP_F8 = ml_dtypes.float8_e4m3
NP_BF = ml_dtypes.bfloat16


def fixup_multi_waits(nc):
    """This toolchain's walrus accepts at most ONE sync wait per instruction;
    Tile emits several. Hoist extras onto same-engine NOPs placed before."""
    for f in nc.m.functions:
        for blk in f.blocks:
            out = []
            changed = False
            for inst in blk.instructions:
                si = getattr(inst, "sync_info", None)
                waits = list(si.on_wait) if si is not None and si.on_wait else []
                if len(waits) > 1:
                    changed = True
                    for k, w in enumerate(waits[:-1]):
                        nop = mybir.InstNoOp(
                            name=f"{inst.name}-waitsplit{k}", ins=[], outs=[]
                        )
                        nop.engine = inst.engine
                        nop.sync_info = mybir.SyncInfo(on_wait=[w], on_update=[])
                        out.append(nop)
                    si.on_wait = [waits[-1]]
                out.append(inst)
            if changed:
                blk.instructions = out


def build(fixup=True):
    nc = bass.Bass(trn_type="TRN2", num_devices=NCORES)

    def din(name, shape, dty):
        return nc.dram_tensor(name, shape, dty, kind="ExternalInput")

    P = 128
    xq = din("xq", [NBT, P, KC, 2, BT], F8)
    hq = din("hq", [NBT, P, KC, 2, BT], F8)
    mq = din("mq", [NBT, P, 2, BT], F8)
    cq = din("cq", [NBT, P, 2, CW], BF16)
    whq = din("whq", [NC, P, KC, 2, CW], F8)
    wxq = din("wxq", [NC, P, KC, 2, CW], F8)
    m3q = din("m3q", [NC, P, 3, 2, CW], F8)
    rowq = din("rowq", [1, 3 * RS], BF16)
    lnwq = din("lnwq", [P, RS], BF16)
    lnbq = din("lnbq", [P, RS], BF16)
    hnq = nc.dram_tensor("hnq", [NBT, P, 2, CW], BF16, kind="ExternalOutput")
    cnq = nc.dram_tensor("cnq", [NBT, P, 2, CW], BF16, kind="ExternalOutput")

    from contextlib import ExitStack

    with tile.TileContext(nc) as tc, ExitStack() as st:
        e = st.enter_context

        class pools:
            wres = e(tc.tile_pool(name="wres", bufs=1))
            psW = e(tc.tile_pool(name="psW", bufs=3, space="PSUM"))
            psD = e(tc.tile_pool(name="psD", bufs=5, space="PSUM"))
            yp = e(tc.tile_pool(name="yp", bufs=16))
            up = e(tc.tile_pool(name="up", bufs=3))
            vp = e(tc.tile_pool(name="vp", bufs=3))
            qp = e(tc.tile_pool(name="qp", bufs=2))
            gp = e(tc.tile_pool(name="gp", bufs=14))
            pb = e(tc.tile_pool(name="pb", bufs=2))
            mp = e(tc.tile_pool(name="mp", bufs=2))

        _emit(nc, pools, locals())

    if fixup:
        fixup_multi_waits(nc)
    return nc


def _emit(nc, pl, d):
    P = 128
    xq, hq, mq, cq = d["xq"], d["hq"], d["mq"], d["cq"]
    whq, wxq, m3q = d["whq"], d["wxq"], d["m3q"]
    rowq, lnwq, lnbq = d["rowq"], d["lnwq"], d["lnbq"]
    hnq, cnq = d["hnq"], d["cnq"]
    wres, psW, psD = pl.wres, pl.psW, pl.psD
    yp, up, vp, qp, gp, pb, mp = pl.yp, pl.up, pl.vp, pl.qp, pl.gp, pl.pb, pl.mp

    # ---- persistent small tiles
    ones = wres.tile([1, P], BF16, name="ones")
    nc.vector.memset(ones[:], 1.0)
    eps_t = wres.tile([P, 1], F32, name="eps")
    nc.vector.memset(eps_t[:], (SM * SW) ** 2 * 1e-5)

    # ---- DMA streams in strict chunk-consumption order, bytes balanced
    # across the two HWDGE queues (SyncE / ScalarE).
    # sync:   rows/rz/idt, wh[c] (+m3[odd c]), bt1 acts, lnw, c, outputs
    # scalar: bt0 acts, wx[c] (+m3[even c]) -- all issued before compute
    rows_t = wres.tile([1, 3 * RS], BF16, name="rows")
    nc.sync.dma_start(rows_t[:], rowq.ap()[:])
    lnw_r = wres.tile([P, NC, CW], BF16, name="lnw_r")
    lnb_r = wres.tile([P, NC, CW], BF16, name="lnb_r")

    xb, hb, mb_, cb = {}, {}, {}, {}

    def act_dmas(bt, eng):
        hb[bt] = wres.tile([P, KC, 2, BT], F8, name=f"hb{bt}")
        eng.dma_start(hb[bt][:], hq.ap()[bt])
        xb[bt] = wres.tile([P, KC, 2, BT], F8, name=f"xb{bt}")
        eng.dma_start(xb[bt][:], xq.ap()[bt])
        mb_[bt] = wres.tile([P, 2, BT], F8, name=f"mb{bt}")
        eng.dma_start(mb_[bt][:], mq.ap()[bt])

    act_dmas(0, nc.scalar)
    act_dmas(1, nc.sync)

    whb, wxb, m3b = {}, {}, {}
    for c in range(NC):
        whb[c] = wres.tile([P, KC, 2, CW], F8, name=f"whb{c}")
        wxb[c] = wres.tile([P, KC, 2, CW], F8, name=f"wxb{c}")
        m3b[c] = wres.tile([P, 3, 2, CW], F8, name=f"m3b{c}")
        if c == 0:
            # k-sliced so the first matmuls can start before the whole
            # chunk lands
            for kc in range(KC):
                nc.sync.dma_start(whb[c][:, kc], whq.ap()[c][:, kc])
                nc.scalar.dma_start(wxb[c][:, kc], wxq.ap()[c][:, kc])
        else:
            nc.sync.dma_start(whb[c][:], whq.ap()[c])
            nc.scalar.dma_start(wxb[c][:], wxq.ap()[c])
        (nc.sync if c % 2 else nc.scalar).dma_start(m3b[c][:], m3q.ap()[c])
    for bt in range(NBT):
        cb[bt] = wres.tile([P, 2, CW], BF16, name=f"cb{bt}")
        nc.sync.dma_start(cb[bt][:], cq.ap()[bt])
    # needed only once phase_b(0) starts (~halfway through): stream last
    nc.sync.dma_start(lnw_r[:], lnwq.ap().rearrange("p (c w) -> p c w", w=CW))
    nc.sync.dma_start(lnb_r[:], lnbq.ap().rearrange("p (c w) -> p c w", w=CW))

    # activation-table preloads: after the DMA issues, before first real use
    dum = wres.tile([P, 1], F32, name="dum")
    for fn in (AF.Square, AF.Sigmoid, AF.Tanh, AF.Sqrt):
        nc.scalar.activation(dum[:], eps_t[:], fn)

    ytiles, gts, moms, rss, nmrss = {}, {}, {}, {}, {}

    def gemm_chunk(bt, c, crit=False):
        WH = psW.tile([P, CW], F32, tag="pw")
        for kc in range(KC):
            nc.tensor.matmul(WH[:], hb[bt][:, kc], whb[c][:, kc],
                             start=(kc == 0), stop=(kc == KC - 1),
                             perf_mode=DR)
        WX = psW.tile([P, CW], F32, tag="pw")
        for kc in range(KC):
            nc.tensor.matmul(WX[:], xb[bt][:, kc], wxb[c][:, kc],
                             start=(kc == 0), stop=(kc == KC - 1),
                             perf_mode=DR)
        Dt = []
        for j in range(3):
            Dj = psD.tile([P, CW], F32, tag="pd")
            nc.tensor.matmul(Dj[:], mb_[bt][:], m3b[c][:, j],
                             start=True, stop=False, perf_mode=DR)
            nc.tensor.matmul(Dj[:], ones[:1, :],
                             rows_t[:1, j * RS + c * CW:j * RS + (c + 1) * CW],
                             start=False, stop=True)
            Dt.append(Dj)
        DH, DX, DB = Dt
        # DVE may read at most ONE input from PSUM per instruction and
        # GpSimd cannot touch PSUM at all: ScalarE stages WH/WX/DB into
        # SBUF, DVE pairs W with D, GpSimd does the SBUF-only add.
        wh_s = up.tile([P, CW], BF16, tag="whs")
        nc.scalar.copy(wh_s[:], WH[:])
        wx_s = vp.tile([P, CW], BF16, tag="wxs")
        nc.scalar.copy(wx_s[:], WX[:])
        u = up.tile([P, CW], BF16, tag="u")
        nc.vector.tensor_mul(u[:], DH[:], wh_s[:])
        v = vp.tile([P, CW], BF16, tag="v")
        nc.vector.tensor_mul(v[:], DX[:], wx_s[:])
        t = up.tile([P, CW], BF16, tag="t")
        (nc.vector if crit else nc.gpsimd).tensor_add(t[:], u[:], v[:])
        y = yp.tile([P, CW], BF16, tag="y")
        nc.vector.scalar_tensor_tensor(y[:], DB[:], 1.0, t[:],
                                       ALU.mult, ALU.add)
        nc.vector.reduce_sum(moms[bt][:, c:c + 1], y[:],
                             axis=mybir.AxisListType.X)
        ysq = qp.tile([P, CW], BF16, tag="ysq")
        nc.scalar.activation(ysq[:], y[:], AF.Square,
                             accum_out=moms[bt][:, 8 + c:9 + c])
        ytiles[(bt, c)] = y

    def mb_start(bt):
        momt = moms[bt]
        S = pb.tile([P, 8], F32, tag="S")
        nc.vector.tensor_add(S[:, 0:4], momt[:, 0:8:2], momt[:, 1:8:2])
        nc.vector.tensor_add(S[:, 4:8], momt[:, 8:16:2], momt[:, 9:16:2])
        scl = pb.tile([P, 8], F32, tag="scl")
        nc.vector.tensor_scalar_mul(scl[:], S[:], 1.0 / H)
        mu = scl[:, 0:4]
        var = pb.tile([P, 4], F32, tag="var")
        nc.vector.tensor_mul(var[:], mu, mu)
        nc.vector.tensor_sub(var[:], scl[:, 4:8], var[:])
        sq = pb.tile([P, 4], F32, tag="sq")
        nc.scalar.activation(sq[:], var[:], AF.Sqrt, bias=eps_t[:])
        rs = pb.tile([P, 4], F32, tag="rs")
        nc.vector.reciprocal(rs[:], sq[:])
        nmrs = pb.tile([P, 4], F32, tag="nmrs")
        nc.vector.scalar_tensor_tensor(nmrs[:], mu, -1.0, rs[:],
                                       ALU.mult, ALU.mult)
        rss[bt], nmrss[bt] = rs, nmrs

    def mb_chunk(bt, c, tail=False):
        g = c // 2
        y = ytiles.pop((bt, c))
        # yn = y*rs + nmrs (cheap tensor_scalar), then *lnw (DVE TT), +lnb
        s0 = pb.tile([P, CW], BF16, tag="s0")
        nc.vector.tensor_scalar(s0[:], y[:], rss[bt][:, g:g + 1],
                                nmrss[bt][:, g:g + 1],
                                op0=ALU.mult, op1=ALU.add)
        w1 = pb.tile([P, CW], BF16, tag="w1")
        nc.vector.tensor_mul(w1[:], s0[:], lnw_r[:, c])
        vv = pb.tile([P, CW], BF16, tag="vv")
        # GpSimd only while the PE still has GEMMs to hide it behind; in
        # the post-GEMM tail DVE is ~3x faster per op
        eng = nc.vector if tail else nc.gpsimd
        eng.tensor_add(vv[:], w1[:], lnb_r[:, c])
        gt_t = gp.tile([P, CW], BF16, tag="gt")
        nc.scalar.activation(gt_t[:], vv[:], AF.Sigmoid if g < 3 else AF.Tanh)
        gts[(bt, c)] = gt_t

    def mb_half(bt, half):
        i_t = gts.pop((bt, 0 + half))
        f_t = gts.pop((bt, 2 + half))
        o_t = gts.pop((bt, 4 + half))
        q_t = gts.pop((bt, 6 + half))
        sfc = pb.tile([P, CW], BF16, tag="sfc")
        nc.gpsimd.tensor_mul(sfc[:], f_t[:], cb[bt][:, half])
        sit = pb.tile([P, CW], BF16, tag="sit")
        nc.vector.tensor_mul(sit[:], i_t[:], q_t[:])
        cnt = pb.tile([P, CW], BF16, tag="cnt")
        nc.vector.tensor_add(cnt[:], sfc[:], sit[:])
        nc.sync.dma_start(cnq.ap()[bt][:, half], cnt[:])
        tct = pb.tile([P, CW], BF16, tag="tct")
        nc.scalar.activation(tct[:], cnt[:], AF.Tanh)
        hnt = pb.tile([P, CW], BF16, tag="hnt")
        nc.gpsimd.tensor_mul(hnt[:], o_t[:], tct[:])
        nc.sync.dma_start(hnq.ap()[bt][:, half], hnt[:])

    # ---- main schedule (hybrid): chunk-PAIRS over the DMA-paced stream-in
    # (both batch tiles consume chunk c back-to-back: ~10us of tensor work
    # per ~5.6us of weight arrival, so the PE never starves and the HAM
    # clock stays at full rate), then bt0 finishes alone, and bt0's
    # epilogue interleaves under bt1's remaining GEMMs. Only bt1's
    # epilogue trails the GEMMs.
    PAIR = 1
    for bt in range(NBT):
        moms[bt] = mp.tile([P, 16], F32, tag="mom", name=f"mom{bt}")
    for c in range(PAIR):
        gemm_chunk(0, c)
        gemm_chunk(1, c)
    for c in range(PAIR, NC):
        gemm_chunk(0, c)
    mb_start(0)
    # spread bt0's 8 chunk-epilogues over bt1's remaining GEMM slots;
    # emit each gate-combine half as soon as its last gate is available
    sched = [1, 1, 1, 1, 1, 1, 2]
    nxt = 0
    for i, c in enumerate(range(PAIR, NC)):
        gemm_chunk(1, c, crit=(c >= NC - 3))
        for _ in range(sched[i]):
            mb_chunk(0, nxt)
            nxt += 1
            if nxt == NC - 1:
                mb_half(0, 0)
    mb_half(0, 1)
    # keep the Sqrt activation table hot for mb_start(1)'s critical path
    nc.scalar.activation(dum[:], eps_t[:], AF.Sqrt)
    mb_start(1)
    for c in range(NC):
        mb_chunk(1, c, tail=True)
        if c == NC - 2:
            mb_half(1, 0)
    mb_half(1, 1)


_nc = None


def _get_nc():
    global _nc
    if _nc is None:
        _nc = build()
    return _nc


def _pack_k(a):
    """[K, C] -> [128, K//256, 2, C] with k = kc*256 + 2p + i (DoubleRow)."""
    K, C = a.shape
    return np.ascontiguousarray(
        a.reshape(K // 256, 128, 2, C).transpose(1, 0, 2, 3))


def _q8(a):
    return np.clip(a, -240.0, 240.0).astype(NP_F8)


def _row_flat(v):
    """[G, H] -> [RS] in chunk-major order (c = g*2 + half)."""
    return np.ascontiguousarray(v.reshape(G * 2, CW).reshape(-1))


def make_in_maps(src_x, h, c, src_meta, zh_w, zh_b, zx_w, zx_b, zb_w,
                 dh_w, dx_w, db_w, db_b, w_h, w_x, ln_w, ln_b):
    f32 = np.float32
    perm = list(PERM)
    w_h = w_h[perm]
    w_x = w_x[perm]
    dh_w = dh_w[perm]
    dx_w = dx_w[perm]
    db_w = db_w[perm]
    db_b = db_b[perm]
    ln_w = ln_w[perm]
    ln_b = ln_b[perm]
    zh3 = zh_w.reshape(G, Z, Z)[perm]
    zx3 = zx_w.reshape(G, Z, Z)[perm]
    zb3 = zb_w.reshape(G, Z, Z)[perm]
    zh_b2 = zh_b.reshape(G, Z)[perm]
    zx_b2 = zx_b.reshape(G, Z)[perm]

    # hypernetwork fold: D_*(b) = src_meta[b] @ M_*[g] + bias row
    M_h = np.einsum("gzy,ghz->gyh", zh3, dh_w).astype(f32)
    M_x = np.einsum("gzy,ghz->gyh", zx3, dx_w).astype(f32)
    M_b = np.einsum("gzy,ghz->gyh", zb3, db_w).astype(f32)
    bdh = np.einsum("gz,ghz->gh", zh_b2, dh_w).astype(f32)
    bdx = np.einsum("gz,ghz->gh", zx_b2, dx_w).astype(f32)

    # replicated (per-core-identical) weight uploads
    def wpack(w):
        out = np.empty((NC, 128, KC, 2, CW), f32)
        for cidx in range(NC):
            g, half = cidx // 2, cidx % 2
            blk = w[g, half * CW:(half + 1) * CW, :]          # [CW, IN]
            out[cidx] = _pack_k(np.ascontiguousarray(blk.T))  # [IN, CW] packed
        return _q8(out * SW)

    def mpack(M, scale):
        out = np.empty((NC, 128, 2, CW), f32)
        for cidx in range(NC):
            g, half = cidx // 2, cidx % 2
            out[cidx] = _pack_k(M[g][:, half * CW:(half + 1) * CW])[:, 0]
        return _q8(out * scale)

    whq = wpack(w_h)
    wxq = wpack(w_x)
    # combined meta-GEMM weights [NC, 128, 3, 2, CW]: j = (h, x, b)
    m3q = np.stack([mpack(M_h, SM), mpack(M_x, SM), mpack(M_b, SM * SW)],
                   axis=2)
    # D_H/D_X rows ride at scale SM (they multiply W at scale SW -> SM*SW);
    # the additive D_B path carries the full SM*SW scale itself.
    rowq = np.concatenate([_row_flat(bdh * SM), _row_flat(bdx * SM),
                           _row_flat((db_b * SM * SW).astype(f32))])[None, :] \
        .astype(NP_BF)
    lnwq = np.ascontiguousarray(
        np.broadcast_to(_row_flat(ln_w)[None, :], (128, RS))).astype(NP_BF)
    lnbq = np.ascontiguousarray(
        np.broadcast_to(_row_flat(ln_b)[None, :], (128, RS))).astype(NP_BF)

    xT = np.ascontiguousarray(src_x.T.astype(f32, copy=False))
    hT = np.ascontiguousarray(h.T.astype(f32, copy=False))
    mT = np.ascontiguousarray(src_meta.T.astype(f32, copy=False))

    in_maps = []
    for ci in range(NCORES):
        r0 = ci * BSH

        def actpack(aT):  # [K, B] slice -> [NBT, 128, K//256, 2, BT] fp8
            out = np.empty((NBT, 128, aT.shape[0] // 256, 2, BT), f32)
            for bt in range(NBT):
                out[bt] = _pack_k(aT[:, r0 + bt * BT:r0 + (bt + 1) * BT])
            return _q8(out)

        c_sl = c[r0:r0 + BSH].reshape(NBT, 128, 2, CW)
        in_maps.append({
            "xq": actpack(xT), "hq": actpack(hT),
            "mq": actpack(mT)[:, :, 0],
            "cq": c_sl.astype(NP_BF),
            "whq": whq, "wxq": wxq, "m3q": m3q,
            "rowq": rowq, "lnwq": lnwq, "lnbq": lnbq,
        })
    return in_maps


def run(inputs, trace=False):
    nc = _get_nc()
    in_maps = make_in_maps(**inputs)
    res = run_bass_kernel_spmd(nc, in_maps, core_ids=list(range(NCORES)),
                               trace=trace)
    h_next = np.empty((B, H), np.float32)
    c_next = np.empty((B, H), np.float32)
    for ci in range(NCORES):
        rows = slice(ci * BSH, (ci + 1) * BSH)
        h_next[rows] = res.results[ci]["hnq"].reshape(BSH, H).astype(np.float32)
        c_next[rows] = res.results[ci]["cnq"].reshape(BSH, H).astype(np.float32)
    return (h_next, c_next), res


def kernel(**inputs):
    (h_next, c_next), _ = run(inputs, trace=False)
    return (h_next, c_next)


# revision 86
# speedup vs baseline: 1.0482x; 1.0482x over previous
"""MetaLSTMCell TRN2 kernel: pure batch-parallel across 8 cores, no collectives.

Each core owns 256 batch rows (2 tiles of 128) and computes the full hidden
dim (4 gates x 1024 cols, as 8 chunks of 512), so the per-gate LayerNorm is
entirely core-local -- no AllReduce, no CC entry barrier (the measured CC
cost in the old hidden-sharded layout was a 44us entry barrier plus
6-44us per tiny AllReduce).

Host-side weight preprocessing: the hypernetwork projections are folded into
M_*[g] = contract(z*_w[g], d*_w[g]) so that d_*(b) = src_meta[b] @ M_*[g] +
bias row, and all large operands are pre-quantized to fp8-e4m3 (TRN variant,
max +-240) with power-of-2 scales (w_h/w_x x32, M_h/M_x x64, M_b x2048).
y rides at a single global scale SM*SW everywhere; LayerNorm is
scale-invariant so no descales are needed (only eps is rescaled).

GEMMs run as fp8 DoubleRow matmuls (K=256 per instruction -- on this part
DR matches bf16 MAC rate but halves instruction count and weight bytes).
Per-gate bias rows are folded in as K=1 bf16 matmuls against a ones row.
Engine split per 512-col chunk (measured ~0.4-0.7us per [128,512] op on
DVE/ScalarE, ~1.2us on GpSimd, which also cannot read PSUM; DVE may read
at most ONE PSUM operand per instruction):
  ScalarE: stage WH/WX PSUM->SBUF + Square(y) with accum (sum of squares)
  DVE:     u=DH*wh_s, v=DX*wx_s, y=DB+t (PSUM readers), row-sum reduce
  GpSimd:  t=u+v (SBUF-only; switched to DVE for the latency-critical
           last chunks and for the post-GEMM tail where DVE is ~3x faster)
Phase_b per chunk: yn=y*rs+nmrs (tensor_scalar), *lnw (TT), +lnb, activation.

Schedule: chunk 0 runs for both batch tiles back-to-back (the weight
stream is the pacer early on; pairing gives ~10us of tensor work per
~5.6us of weight arrival so the PE array never idles and its HAM clock
gate stays at full rate), then bt0 finishes alone, and bt0's epilogue is
spread under bt1's remaining GEMM slots. Only bt1's epilogue trails.

DMA: weights stream on both HWDGE queues in consumption order (wh on
SyncE, wx on ScalarE, m3 alternating; first chunk k-sliced), activations
first, LN tables last; outputs go out on the (by then idle) sync queue.
Activation tables are preloaded with dummy ops during the ramp.
"""

import sys

sys.path.insert(0, "/opt/trn_rl_repo")

import ml_dtypes
import numpy as np
import concourse.bass as bass
import concourse.mybir as mybir
from concourse.bass_utils import run_bass_kernel_spmd
import concourse.tile as tile

B, IN, H, Z, G = 2048, 1024, 1024, 256, 4
NCORES = 8
BSH = B // NCORES          # 256 batch rows per core
BT = 128                   # batch tile (PE output partitions)
NBT = BSH // BT            # 2 batch tiles per core
CW = 512                   # column chunk width
NC = G * H // CW           # 8 chunks; chunk c = (gate g=c//2, half=c%2)
KC = IN // 256             # 4 DoubleRow K-chunks for the main GEMMs
RS = NC * CW               # 4096: one bias row
SM, SW = 64.0, 32.0        # fp8 pre-scales for M_h/M_x and w_h/w_x
PERM = (0, 1, 3, 2)        # gate order [i, f, o, g]

dt = mybir.dt
AF = mybir.ActivationFunctionType
ALU = mybir.AluOpType
DR = mybir.MatmulPerfMode.DoubleRow
F32, BF16, F8 = dt.float32, dt.bfloat16, dt.float8e4

NP_F8 = ml_dtypes.float8_e4m3
NP_BF = ml_dtypes.bfloat16


def fixup_multi_waits(nc):
    """This toolchain's walrus accepts at most ONE sync wait per instruction;
    Tile emits several. Hoist extras onto same-engine NOPs placed before."""
    for f in nc.m.functions:
        for blk in f.blocks:
            out = []
            changed = False
            for inst in blk.instructions:
                si = getattr(inst, "sync_info", None)
                waits = list(si.on_wait) if si is not None and si.on_wait else []
                if len(waits) > 1:
                    changed = True
                    for k, w in enumerate(waits[:-1]):
                        nop = mybir.InstNoOp(
                            name=f"{inst.name}-waitsplit{k}", ins=[], outs=[]
                        )
                        nop.engine = inst.engine
                        nop.sync_info = mybir.SyncInfo(on_wait=[w], on_update=[])
                        out.append(nop)
                    si.on_wait = [waits[-1]]
                out.append(inst)
            if changed:
                blk.instructions = out


def build(fixup=True):
    nc = bass.Bass(trn_type="TRN2", num_devices=NCORES)

    def din(name, shape, dty):
        return nc.dram_tensor(name, shape, dty, kind="ExternalInput")

    P = 128
    xq = din("xq", [NBT, P, KC, 2, BT], F8)
    hq = din("hq", [NBT, P, KC, 2, BT], F8)
    mq = din("mq", [NBT, P, 2, BT], F8)
    cq = din("cq", [NBT, P, 2, CW], BF16)
    whq = din("whq", [NC, P, KC, 2, CW], F8)
    wxq = din("wxq", [NC, P, KC, 2, CW], F8)
    m3q = din("m3q", [NC, P, 3, 2, CW], F8)
    rowq = din("rowq", [1, 3 * RS], BF16)
    lnwq = din("lnwq", [P, RS], BF16)
    lnbq = din("lnbq", [P, RS], BF16)
    hnq = nc.dram_tensor("hnq", [NBT, P, 2, CW], BF16, kind="ExternalOutput")
    cnq = nc.dram_tensor("cnq", [NBT, P, 2, CW], BF16, kind="ExternalOutput")

    from contextlib import ExitStack

    with tile.TileContext(nc) as tc, ExitStack() as st:
        e = st.enter_context

        class pools:
            wres = e(tc.tile_pool(name="wres", bufs=1))
            psW = e(tc.tile_pool(name="psW", bufs=3, space="PSUM"))
            psD = e(tc.tile_pool(name="psD", bufs=5, space="PSUM"))
            yp = e(tc.tile_pool(name="yp", bufs=16))
            up = e(tc.tile_pool(name="up", bufs=3))
            vp = e(tc.tile_pool(name="vp", bufs=3))
            qp = e(tc.tile_pool(name="qp", bufs=2))
            gp = e(tc.tile_pool(name="gp", bufs=14))
            pb = e(tc.tile_pool(name="pb", bufs=2))
            mp = e(tc.tile_pool(name="mp", bufs=2))

        _emit(nc, pools, locals())

    if fixup:
        fixup_multi_waits(nc)
    return nc


def _emit(nc, pl, d):
    P = 128
    xq, hq, mq, cq = d["xq"], d["hq"], d["mq"], d["cq"]
    whq, wxq, m3q = d["whq"], d["wxq"], d["m3q"]
    rowq, lnwq, lnbq = d["rowq"], d["lnwq"], d["lnbq"]
    hnq, cnq = d["hnq"], d["cnq"]
    wres, psW, psD = pl.wres, pl.psW, pl.psD
    yp, up, vp, qp, gp, pb, mp = pl.yp, pl.up, pl.vp, pl.qp, pl.gp, pl.pb, pl.mp

    # ---- persistent small tiles
    ones = wres.tile([1, P], BF16, name="ones")
    nc.vector.memset(ones[:], 1.0)
    eps_t = wres.tile([P, 1], F32, name="eps")
    nc.vector.memset(eps_t[:], (SM * SW) ** 2 * 1e-5)

    # ---- DMA streams in strict chunk-consumption order, bytes balanced
    # across the two HWDGE queues (SyncE / ScalarE).
    # sync:   rows/rz/idt, wh[c] (+m3[odd c]), bt1 acts, lnw, c, outputs
    # scalar: bt0 acts, wx[c] (+m3[even c]) -- all issued before compute
    rows_t = wres.tile([1, 3 * RS], BF16, name="rows")
    nc.sync.dma_start(rows_t[:], rowq.ap()[:])
    lnw_r = wres.tile([P, NC, CW], BF16, name="lnw_r")
    lnb_r = wres.tile([P, NC, CW], BF16, name="lnb_r")

    xb, hb, mb_, cb = {}, {}, {}, {}

    def act_dmas(bt, eng):
        hb[bt] = wres.tile([P, KC, 2, BT], F8, name=f"hb{bt}")
        eng.dma_start(hb[bt][:], hq.ap()[bt])
        xb[bt] = wres.tile([P, KC, 2, BT], F8, name=f"xb{bt}")
        eng.dma_start(xb[bt][:], xq.ap()[bt])
        mb_[bt] = wres.tile([P, 2, BT], F8, name=f"mb{bt}")
        eng.dma_start(mb_[bt][:], mq.ap()[bt])

    act_dmas(0, nc.scalar)
    act_dmas(1, nc.sync)

    whb, wxb, m3b = {}, {}, {}
    for c in range(NC):
        whb[c] = wres.tile([P, KC, 2, CW], F8, name=f"whb{c}")
        wxb[c] = wres.tile([P, KC, 2, CW], F8, name=f"wxb{c}")
        m3b[c] = wres.tile([P, 3, 2, CW], F8, name=f"m3b{c}")
        if c == 0:
            # k-sliced so the first matmuls can start before the whole
            # chunk lands
            for kc in range(KC):
                nc.sync.dma_start(whb[c][:, kc], whq.ap()[c][:, kc])
                nc.scalar.dma_start(wxb[c][:, kc], wxq.ap()[c][:, kc])
        else:
            nc.sync.dma_start(whb[c][:], whq.ap()[c])
            nc.scalar.dma_start(wxb[c][:], wxq.ap()[c])
        (nc.sync if c % 2 else nc.scalar).dma_start(m3b[c][:], m3q.ap()[c])
    for bt in range(NBT):
        cb[bt] = wres.tile([P, 2, CW], BF16, name=f"cb{bt}")
        nc.sync.dma_start(cb[bt][:], cq.ap()[bt])
    # needed only once phase_b(0) starts (~halfway through): stream last
    nc.sync.dma_start(lnw_r[:], lnwq.ap().rearrange("p (c w) -> p c w", w=CW))
    nc.sync.dma_start(lnb_r[:], lnbq.ap().rearrange("p (c w) -> p c w", w=CW))

    # activation-table preloads: after the DMA issues, before first real use
    dum = wres.tile([P, 1], F32, name="dum")
    for fn in (AF.Square, AF.Sigmoid, AF.Tanh, AF.Sqrt):
        nc.scalar.activation(dum[:], eps_t[:], fn)

    ytiles, gts, moms, rss, nmrss = {}, {}, {}, {}, {}

    def gemm_chunk(bt, c, crit=False):
        WH = psW.tile([P, CW], F32, tag="pw")
        for kc in range(KC):
            nc.tensor.matmul(WH[:], hb[bt][:, kc], whb[c][:, kc],
                             start=(kc == 0), stop=(kc == KC - 1),
                             perf_mode=DR)
        WX = psW.tile([P, CW], F32, tag="pw")
        for kc in range(KC):
            nc.tensor.matmul(WX[:], xb[bt][:, kc], wxb[c][:, kc],
                             start=(kc == 0), stop=(kc == KC - 1),
                             perf_mode=DR)
        Dt = []
        for j in range(3):
            Dj = psD.tile([P, CW], F32, tag="pd")
            nc.tensor.matmul(Dj[:], mb_[bt][:], m3b[c][:, j],
                             start=True, stop=False, perf_mode=DR)
            nc.tensor.matmul(Dj[:], ones[:1, :],
                             rows_t[:1, j * RS + c * CW:j * RS + (c + 1) * CW],
                             start=False, stop=True)
            Dt.append(Dj)
        DH, DX, DB = Dt
        # DVE may read at most ONE input from PSUM per instruction and
        # GpSimd cannot touch PSUM at all: ScalarE stages WH/WX/DB into
        # SBUF, DVE pairs W with D, GpSimd does the SBUF-only add.
        wh_s = up.tile([P, CW], BF16, tag="whs")
        nc.scalar.copy(wh_s[:], WH[:])
        wx_s = vp.tile([P, CW], BF16, tag="wxs")
        nc.scalar.copy(wx_s[:], WX[:])
        u = up.tile([P, CW], BF16, tag="u")
        nc.vector.tensor_mul(u[:], DH[:], wh_s[:])
        v = vp.tile([P, CW], BF16, tag="v")
        nc.vector.tensor_mul(v[:], DX[:], wx_s[:])
        t = up.tile([P, CW], BF16, tag="t")
        (nc.vector if crit else nc.gpsimd).tensor_add(t[:], u[:], v[:])
        y = yp.tile([P, CW], BF16, tag="y")
        nc.vector.scalar_tensor_tensor(y[:], DB[:], 1.0, t[:],
                                       ALU.mult, ALU.add)
        nc.vector.reduce_sum(moms[bt][:, c:c + 1], y[:],
                             axis=mybir.AxisListType.X)
        ysq = qp.tile([P, CW], BF16, tag="ysq")
        nc.scalar.activation(ysq[:], y[:], AF.Square,
                             accum_out=moms[bt][:, 8 + c:9 + c])
        ytiles[(bt, c)] = y

    def mb_start(bt):
        momt = moms[bt]
        S = pb.tile([P, 8], F32, tag="S")
        nc.vector.tensor_add(S[:, 0:4], momt[:, 0:8:2], momt[:, 1:8:2])
        nc.vector.tensor_add(S[:, 4:8], momt[:, 8:16:2], momt[:, 9:16:2])
        scl = pb.tile([P, 8], F32, tag="scl")
        nc.vector.tensor_scalar_mul(scl[:], S[:], 1.0 / H)
        mu = scl[:, 0:4]
        var = pb.tile([P, 4], F32, tag="var")
        nc.vector.tensor_mul(var[:], mu, mu)
        nc.vector.tensor_sub(var[:], scl[:, 4:8], var[:])
        sq = pb.tile([P, 4], F32, tag="sq")
        nc.scalar.activation(sq[:], var[:], AF.Sqrt, bias=eps_t[:])
        rs = pb.tile([P, 4], F32, tag="rs")
        nc.vector.reciprocal(rs[:], sq[:])
        nmrs = pb.tile([P, 4], F32, tag="nmrs")
        nc.vector.scalar_tensor_tensor(nmrs[:], mu, -1.0, rs[:],
                                       ALU.mult, ALU.mult)
        rss[bt], nmrss[bt] = rs, nmrs

    def mb_chunk(bt, c, tail=False):
        g = c // 2
        y = ytiles.pop((bt, c))
        # yn = y*rs + nmrs (cheap tensor_scalar), then *lnw (DVE TT), +lnb
        s0 = pb.tile([P, CW], BF16, tag="s0")
        nc.vector.tensor_scalar(s0[:], y[:], rss[bt][:, g:g + 1],
                                nmrss[bt][:, g:g + 1],
                                op0=ALU.mult, op1=ALU.add)
        w1 = pb.tile([P, CW], BF16, tag="w1")
        nc.vector.tensor_mul(w1[:], s0[:], lnw_r[:, c])
        vv = pb.tile([P, CW], BF16, tag="vv")
        # GpSimd only while the PE still has GEMMs to hide it behind; in
        # the post-GEMM tail DVE is ~3x faster per op
        eng = nc.vector if tail else nc.gpsimd
        eng.tensor_add(vv[:], w1[:], lnb_r[:, c])
        gt_t = gp.tile([P, CW], BF16, tag="gt")
        nc.scalar.activation(gt_t[:], vv[:], AF.Sigmoid if g < 3 else AF.Tanh)
        gts[(bt, c)] = gt_t

    def mb_half(bt, half):
        i_t = gts.pop((bt, 0 + half))
        f_t = gts.pop((bt, 2 + half))
        o_t = gts.pop((bt, 4 + half))
        q_t = gts.pop((bt, 6 + half))
        # bt1's halves sit on the kernel's end-chain: keep them off the
        # slow GpSimd entirely
        eng = nc.vector if bt == 1 else nc.gpsimd
        sfc = pb.tile([P, CW], BF16, tag="sfc")
        eng.tensor_mul(sfc[:], f_t[:], cb[bt][:, half])
        sit = pb.tile([P, CW], BF16, tag="sit")
        nc.vector.tensor_mul(sit[:], i_t[:], q_t[:])
        cnt = pb.tile([P, CW], BF16, tag="cnt")
        nc.vector.tensor_add(cnt[:], sfc[:], sit[:])
        nc.sync.dma_start(cnq.ap()[bt][:, half], cnt[:])
        tct = pb.tile([P, CW], BF16, tag="tct")
        nc.scalar.activation(tct[:], cnt[:], AF.Tanh)
        hnt = pb.tile([P, CW], BF16, tag="hnt")
        eng.tensor_mul(hnt[:], o_t[:], tct[:])
        nc.sync.dma_start(hnq.ap()[bt][:, half], hnt[:])

    # ---- main schedule (hybrid): chunk-PAIRS over the DMA-paced stream-in
    # (both batch tiles consume chunk c back-to-back: ~10us of tensor work
    # per ~5.6us of weight arrival, so the PE never starves and the HAM
    # clock stays at full rate), then bt0 finishes alone, and bt0's
    # epilogue interleaves under bt1's remaining GEMMs. Only bt1's
    # epilogue trails the GEMMs.
    PAIR = 1
    for bt in range(NBT):
        moms[bt] = mp.tile([P, 16], F32, tag="mom", name=f"mom{bt}")
    for c in range(PAIR):
        gemm_chunk(0, c)
        gemm_chunk(1, c)
    for c in range(PAIR, NC):
        gemm_chunk(0, c)
    mb_start(0)
    # spread bt0's 8 chunk-epilogues over bt1's remaining GEMM slots;
    # emit each gate-combine half as soon as its last gate is available
    sched = [1, 1, 1, 2, 1, 1, 1]
    nxt = 0
    for i, c in enumerate(range(PAIR, NC)):
        gemm_chunk(1, c, crit=(c >= NC - 3))
        for _ in range(sched[i]):
            mb_chunk(0, nxt)
            nxt += 1
            if nxt == NC - 1:
                mb_half(0, 0)
    mb_half(0, 1)
    # keep the Sqrt activation table hot for mb_start(1)'s critical path
    nc.scalar.activation(dum[:], eps_t[:], AF.Sqrt)
    mb_start(1)
    for c in range(NC):
        mb_chunk(1, c, tail=True)
        if c == NC - 2:
            mb_half(1, 0)
    mb_half(1, 1)


_nc = None


def _get_nc():
    global _nc
    if _nc is None:
        _nc = build()
    return _nc


def _pack_k(a):
    """[K, C] -> [128, K//256, 2, C] with k = kc*256 + 2p + i (DoubleRow)."""
    K, C = a.shape
    return np.ascontiguousarray(
        a.reshape(K // 256, 128, 2, C).transpose(1, 0, 2, 3))


def _q8(a):
    return np.clip(a, -240.0, 240.0).astype(NP_F8)


def _row_flat(v):
    """[G, H] -> [RS] in chunk-major order (c = g*2 + half)."""
    return np.ascontiguousarray(v.reshape(G * 2, CW).reshape(-1))


def make_in_maps(src_x, h, c, src_meta, zh_w, zh_b, zx_w, zx_b, zb_w,
                 dh_w, dx_w, db_w, db_b, w_h, w_x, ln_w, ln_b):
    f32 = np.float32
    perm = list(PERM)
    w_h = w_h[perm]
    w_x = w_x[perm]
    dh_w = dh_w[perm]
    dx_w = dx_w[perm]
    db_w = db_w[perm]
    db_b = db_b[perm]
    ln_w = ln_w[perm]
    ln_b = ln_b[perm]
    zh3 = zh_w.reshape(G, Z, Z)[perm]
    zx3 = zx_w.reshape(G, Z, Z)[perm]
    zb3 = zb_w.reshape(G, Z, Z)[perm]
    zh_b2 = zh_b.reshape(G, Z)[perm]
    zx_b2 = zx_b.reshape(G, Z)[perm]

    # hypernetwork fold: D_*(b) = src_meta[b] @ M_*[g] + bias row
    M_h = np.einsum("gzy,ghz->gyh", zh3, dh_w).astype(f32)
    M_x = np.einsum("gzy,ghz->gyh", zx3, dx_w).astype(f32)
    M_b = np.einsum("gzy,ghz->gyh", zb3, db_w).astype(f32)
    bdh = np.einsum("gz,ghz->gh", zh_b2, dh_w).astype(f32)
    bdx = np.einsum("gz,ghz->gh", zx_b2, dx_w).astype(f32)

    # replicated (per-core-identical) weight uploads
    def wpack(w):
        out = np.empty((NC, 128, KC, 2, CW), f32)
        for cidx in range(NC):
            g, half = cidx // 2, cidx % 2
            blk = w[g, half * CW:(half + 1) * CW, :]          # [CW, IN]
            out[cidx] = _pack_k(np.ascontiguousarray(blk.T))  # [IN, CW] packed
        return _q8(out * SW)

    def mpack(M, scale):
        out = np.empty((NC, 128, 2, CW), f32)
        for cidx in range(NC):
            g, half = cidx // 2, cidx % 2
            out[cidx] = _pack_k(M[g][:, half * CW:(half + 1) * CW])[:, 0]
        return _q8(out * scale)

    whq = wpack(w_h)
    wxq = wpack(w_x)
    # combined meta-GEMM weights [NC, 128, 3, 2, CW]: j = (h, x, b)
    m3q = np.stack([mpack(M_h, SM), mpack(M_x, SM), mpack(M_b, SM * SW)],
                   axis=2)
    # D_H/D_X rows ride at scale SM (they multiply W at scale SW -> SM*SW);
    # the additive D_B path carries the full SM*SW scale itself.
    rowq = np.concatenate([_row_flat(bdh * SM), _row_flat(bdx * SM),
                           _row_flat((db_b * SM * SW).astype(f32))])[None, :] \
        .astype(NP_BF)
    lnwq = np.ascontiguousarray(
        np.broadcast_to(_row_flat(ln_w)[None, :], (128, RS))).astype(NP_BF)
    lnbq = np.ascontiguousarray(
        np.broadcast_to(_row_flat(ln_b)[None, :], (128, RS))).astype(NP_BF)

    xT = np.ascontiguousarray(src_x.T.astype(f32, copy=False))
    hT = np.ascontiguousarray(h.T.astype(f32, copy=False))
    mT = np.ascontiguousarray(src_meta.T.astype(f32, copy=False))

    in_maps = []
    for ci in range(NCORES):
        r0 = ci * BSH

        def actpack(aT):  # [K, B] slice -> [NBT, 128, K//256, 2, BT] fp8
            out = np.empty((NBT, 128, aT.shape[0] // 256, 2, BT), f32)
            for bt in range(NBT):
                out[bt] = _pack_k(aT[:, r0 + bt * BT:r0 + (bt + 1) * BT])
            return _q8(out)

        c_sl = c[r0:r0 + BSH].reshape(NBT, 128, 2, CW)
        in_maps.append({
            "xq": actpack(xT), "hq": actpack(hT),
            "mq": actpack(mT)[:, :, 0],
            "cq": c_sl.astype(NP_BF),
            "whq": whq, "wxq": wxq, "m3q": m3q,
            "rowq": rowq, "lnwq": lnwq, "lnbq": lnbq,
        })
    return in_maps


def run(inputs, trace=False):
    nc = _get_nc()
    in_maps = make_in_maps(**inputs)
    res = run_bass_kernel_spmd(nc, in_maps, core_ids=list(range(NCORES)),
                               trace=trace)
    h_next = np.empty((B, H), np.float32)
    c_next = np.empty((B, H), np.float32)
    for ci in range(NCORES):
        rows = slice(ci * BSH, (ci + 1) * BSH)
        h_next[rows] = res.results[ci]["hnq"].reshape(BSH, H).astype(np.float32)
        c_next[rows] = res.results[ci]["cnq"].reshape(BSH, H).astype(np.float32)
    return (h_next, c_next), res


def kernel(**inputs):
    (h_next, c_next), _ = run(inputs, trace=False)
    return (h_next, c_next)
